# revision 19
# baseline (speedup 1.0000x reference)
"""Llama decoder layer on 8 TRN2 NeuronCores, tensor-parallel over heads.

Core c owns q-heads 4c..4c+3 (one GQA group -> kv head c); wq/wk/wv and
gate/up are column-sharded, wo/down row-sharded; partial sums cross cores via
two bf16 AllReduces (chunked by hid for pipelining).  On-device activations
live transposed ([feature, seq]); the host transposes inputs/outputs and folds
ln weights + 1/sqrt(D) into the projections.  The RMSNorm rsqrt scale is
per-sequence-position, so it commutes past the hid-contraction: raw x feeds
the matmuls and the scale fuses into the PSUM->SBUF copies.  Attention path is
float32r (full PE rate, ~fp32 precision); MLP runs bf16.
"""
import sys, os
sys.path.insert(0, '/opt/trn_rl_repo')
import numpy as np
import concourse.bacc as bacc
import concourse.mybir as mybir
import concourse.tile as tile
from concourse.bass_utils import run_bass_kernel_spmd
from concourse.masks import make_identity

F32 = mybir.dt.float32
F32R = mybir.dt.float32r
BF16 = mybir.dt.bfloat16
AF = mybir.ActivationFunctionType
OP = mybir.AluOpType
AX = mybir.AxisListType

NCORES = 8
S = 2048
HID = 2048
D = 64
NQ = 4            # q heads per core
FQ = NQ * D       # 256 q features per core
INT_SH = 1024     # intermediate shard per core
NT = HID // 128   # 16 hid tiles
ST = S // 128     # 16 seq tiles
SC = S // 512     # 4 seq chunks
EPS = 1e-6
ROPE_THETA = 10000.0

_CACHED = {}


def _build():
    nc = bacc.Bacc("TRN2", target_bir_lowering=False, num_devices=NCORES)
    _eps_t = nc.alloc_sbuf_tensor("const-eps", [128, 1], F32)
    nc.gpsimd.memset(_eps_t.ap(), EPS)
    nc.const_aps.aps[(F32, EPS)] = _eps_t.ap()
    nc.all_engine_barrier()

    xT = nc.dram_tensor("xT", [HID, S], F32, kind="ExternalInput")
    wqT = nc.dram_tensor("wqT", [HID, FQ], F32, kind="ExternalInput")
    wkvT = nc.dram_tensor("wkvT", [HID, 128], F32, kind="ExternalInput")
    woT = nc.dram_tensor("woT", [FQ, HID], F32, kind="ExternalInput")
    wgT = nc.dram_tensor("wgT", [HID, INT_SH], BF16, kind="ExternalInput")
    wuT = nc.dram_tensor("wuT", [HID, INT_SH], BF16, kind="ExternalInput")
    wdT = nc.dram_tensor("wdT", [INT_SH, HID], BF16, kind="ExternalInput")
    cos2 = nc.dram_tensor("cos2", [128, S], F32, kind="ExternalInput")
    sins2 = nc.dram_tensor("sins2", [128, S], F32, kind="ExternalInput")
    maskN = nc.dram_tensor("maskN", [512, 512], F32, kind="ExternalInput")
    maskT = nc.dram_tensor("maskT", [512, 512], F32, kind="ExternalInput")
    ones_in = nc.dram_tensor("ones_in", [128, 1], F32, kind="ExternalInput")
    ones_row_in = nc.dram_tensor("ones_row", [1, 128], F32, kind="ExternalInput")

    attn_w = nc.dram_tensor("attn_w", [NQ, S, S], F32, kind="ExternalOutput")
    o_attn = nc.dram_tensor("o_attn", [HID, S], BF16, kind="ExternalOutput")
    o_mlp = nc.dram_tensor("o_mlp", [HID, S], BF16, kind="ExternalOutput")

    def f32r(ap):
        return ap.bitcast(F32R)

    RG = [list(range(NCORES))]

    with tile.TileContext(nc) as tc:
        with tc.tile_pool(name="dram", bufs=1, space="DRAM") as dr, \
             tc.tile_pool(name="cpool", bufs=1) as cpool, \
             tc.tile_pool(name="psmm", bufs=5, space="PSUM") as ps, \
             tc.tile_pool(name="psav", bufs=2, space="PSUM") as psav, \
             tc.tile_pool(name="psq", bufs=1, space="PSUM") as psq:

            ar1_in = [dr.tile([HID, 512], BF16, name=f"ar1i{g}")
                      for g in range(4)]
            ar1_out = [dr.tile([HID, 512], BF16, addr_space="Shared",
                               name=f"ar1o{g}") for g in range(4)]
            ar2_in = [dr.tile([HID, 512], BF16, name=f"ar2i{g}")
                      for g in range(4)]
            ar2_out = [dr.tile([HID, 512], BF16, addr_space="Shared",
                               name=f"ar2o{g}") for g in range(4)]
            rrow_dr = dr.tile([NQ, S], F32)

            ones_t = cpool.tile([128, 1], F32R, name="ones_t")
            nc.sync.dma_start(out=ones_t[:], in_=f32r(ones_in[:]))
            ones_r = cpool.tile([1, 128], F32R, name="ones_r")
            nc.sync.dma_start(out=ones_r[:], in_=f32r(ones_row_in[:]))
            ident = cpool.tile([128, 128], F32, name="ident")
            make_identity(nc, ident[:])

            # ================= attention super-phase =================
            with tc.tile_pool(name="attp", bufs=1) as ap_:
                qT = [ap_.tile([128, S], F32R, name=f"qT{i}") for i in range(2)]
                kvT = ap_.tile([128, S], F32R, name="kvT")
                kkT = ap_.tile([128, S], F32R, name="kkT")
                vN = ap_.tile([128, ST * D], BF16, name="vN")
                aoT = [[ap_.tile([128, 512], F32R, name=f"aoT{i}_{j}")
                        for j in range(SC)] for i in range(2)]

                # ---- norm1 stats + QKV ----
                with tc.tile_pool(name="xsp", bufs=20) as xsp, \
                     tc.tile_pool(name="sqp", bufs=3) as sqp, \
                     tc.tile_pool(name="wqp", bufs=1) as wqp, \
                     tc.tile_pool(name="ropep", bufs=1) as rp, \
                     tc.tile_pool(name="rotp", bufs=3) as rotp, \
                     tc.tile_pool(name="rowp", bufs=4) as rowp:
                    cos_t = rp.tile([128, S], F32, name="cos_t")
                    nc.sync.dma_start(out=cos_t[:], in_=cos2[:])
                    sin_t = rp.tile([128, S], F32, name="sin_t")
                    nc.sync.dma_start(out=sin_t[:], in_=sins2[:])
                    wq_t = [wqp.tile([128, FQ], F32R, name=f"wq{i}")
                            for i in range(NT)]
                    wkv_t = [wqp.tile([128, 128], F32R, name=f"wkv{i}")
                             for i in range(NT)]
                    for sn in range(SC):
                        sl = slice(sn * 512, (sn + 1) * 512)
                        ssq = psq.tile([1, 512], F32, tag="ssq", name=f"ssq{sn}")
                        xrow = []
                        for i in range(NT):
                            xt = xsp.tile([128, 512], F32R, name=f"x_{sn}_{i}",
                                          tag="xs")
                            nc.sync.dma_start(
                                out=xt[:],
                                in_=f32r(xT[i * 128:(i + 1) * 128,
                                            sn * 512:(sn + 1) * 512]))
                            xrow.append(xt)
                            sq = sqp.tile([128, 512], F32R, tag="sq")
                            nc.vector.tensor_tensor(sq[:], xt[:], xt[:], OP.mult)
                            nc.tensor.matmul(ssq[:], f32r(ones_t[:]), sq[:],
                                             start=(i == 0), stop=(i == NT - 1))
                        if sn == 0:
                            for i in range(NT):
                                nc.sync.dma_start(
                                    out=wq_t[i][:],
                                    in_=f32r(wqT[i * 128:(i + 1) * 128, :]))
                                nc.sync.dma_start(
                                    out=wkv_t[i][:],
                                    in_=f32r(wkvT[i * 128:(i + 1) * 128, :]))
                        srow = rowp.tile([1, 512], F32, tag="srow")
                        nc.scalar.activation(srow[:], ssq[:], AF.Sqrt,
                                             bias=EPS, scale=1.0 / HID)
                        rrow = rowp.tile([1, 512], F32R, tag="rrow")
                        with nc.allow_low_precision(reason="f32r rstd bcast"):
                            nc.vector.reciprocal(rrow[:], srow[:])
                        rbp_ps = psq.tile([128, 512], F32, tag="ssq",
                                          name=f"rbcp{sn}")
                        nc.tensor.matmul(rbp_ps[:], ones_r[:], rrow[:],
                                         start=True, stop=True)
                        rbc = rowp.tile([128, 512], F32, tag="rbc",
                                        name=f"rbc{sn}")
                        nc.scalar.copy(rbc[:], rbp_ps[:])
                        for fc in range(2):
                            pst = ps.tile([128, 512], F32, tag="mm")
                            for i in range(NT):
                                nc.tensor.matmul(
                                    pst[:], wq_t[i][:, fc * 128:(fc + 1) * 128],
                                    xrow[i][:],
                                    start=(i == 0), stop=(i == NT - 1))
                            nc.vector.tensor_tensor(qT[fc][:, sl], f32r(pst[:]),
                                                    f32r(rbc[:]), OP.mult)
                        pst = ps.tile([128, 512], F32, tag="mm")
                        for i in range(NT):
                            nc.tensor.matmul(pst[:], wkv_t[i][:], xrow[i][:],
                                             start=(i == 0), stop=(i == NT - 1))
                        nc.vector.tensor_tensor(kvT[:, sl], f32r(pst[:]),
                                                f32r(rbc[:]), OP.mult)
                        # rope on this s-chunk
                        for tq in range(3):
                            tgt = qT[tq] if tq < 2 else kvT
                            hi = 128 if tq < 2 else 64
                            rot = rotp.tile([128, 512], F32R, tag="rot")
                            for b in range(0, hi, 64):
                                nc.vector.tensor_copy(
                                    rot[b:b + 32, :], tgt[b + 32:b + 64, sl])
                                nc.vector.tensor_copy(
                                    rot[b + 32:b + 64, :], tgt[b:b + 32, sl])
                            nc.vector.tensor_tensor(tgt[0:hi, sl], tgt[0:hi, sl],
                                                    f32r(cos_t[0:hi, sl]),
                                                    OP.mult)
                            nc.vector.tensor_tensor(rot[0:hi, :], rot[0:hi, :],
                                                    f32r(sin_t[0:hi, sl]),
                                                    OP.mult)
                            nc.vector.tensor_tensor(tgt[0:hi, sl], tgt[0:hi, sl],
                                                    rot[0:hi, :], OP.add)
                        nc.vector.tensor_copy(kkT[0:64, sl], kvT[0:64, sl])
                        nc.vector.tensor_copy(kkT[64:128, sl], kvT[0:64, sl])
                        vtmp = rotp.tile([64, 512], F32R, tag="vtmp")
                        nc.vector.tensor_copy(vtmp[:], kvT[64:128, sl])
                        for stl in range(4):
                            st = sn * 4 + stl
                            pst = ps.tile([128, 64], F32, tag="mm")
                            nc.tensor.transpose(
                                pst[:],
                                vtmp[:, stl * 128:(stl + 1) * 128].bitcast(F32),
                                ident[0:64, 0:64])
                            nc.scalar.copy(vN[:, st * 64:(st + 1) * 64], pst[:])

                # ---- attention ----
                with tc.tile_pool(name="mskp", bufs=1) as mp_, \
                     tc.tile_pool(name="pnp", bufs=3) as pnp, \
                     tc.tile_pool(name="expp", bufs=6) as expp, \
                     tc.tile_pool(name="rbp", bufs=4) as rbp, \
                     tc.tile_pool(name="rowp2", bufs=4) as rowp2:
                    mN = []
                    mT = []
                    for m in range(4):
                        t1 = mp_.tile([128, 512], F32, name=f"mN{m}")
                        nc.sync.dma_start(out=t1[:],
                                          in_=maskN[m * 128:(m + 1) * 128, :])
                        mN.append(t1)
                        t2 = mp_.tile([128, 512], F32, name=f"mT{m}")
                        nc.sync.dma_start(out=t2[:],
                                          in_=maskT[m * 128:(m + 1) * 128, :])
                        mT.append(t2)

                    for j in range(SC):
                        jsl = slice(j * 512, (j + 1) * 512)
                        # normal pass: attn_w rows + 1/sumexp
                        for i in range(4 * j, 4 * j + 4):
                            nchunks = i // 4 + 1
                            for h in range(NQ):
                                p = h // 2
                                hb = (h % 2) * 64
                                pn = pnp.tile([128, S], F32, tag="pn")
                                accs = rowp2.tile([128, 4], F32, tag="accs")
                                for n in range(nchunks):
                                    w = min(512, (i + 1) * 128 - n * 512)
                                    pst = ps.tile([128, 512], F32, tag="mm")
                                    nc.tensor.matmul(
                                        pst[:, 0:w],
                                        qT[p][hb:hb + 64, i * 128:(i + 1) * 128],
                                        kkT[hb:hb + 64, n * 512:n * 512 + w],
                                        start=True, stop=True,
                                        tile_position=(hb, 0))
                                    if n == i // 4:
                                        m0 = (i % 4) * 128
                                        nc.vector.tensor_tensor(
                                            pst[:, m0:w], pst[:, m0:w],
                                            mN[i % 4][:, m0:w], OP.add)
                                    nc.scalar.activation(
                                        pn[:, n * 512:n * 512 + w],
                                        pst[:, 0:w], AF.Exp,
                                        accum_out=accs[:, n:n + 1])
                                se = rowp2.tile([128, 1], F32, tag="se")
                                if nchunks == 1:
                                    nc.vector.reciprocal(se[:], accs[:, 0:1])
                                else:
                                    nc.vector.reduce_sum(se[:],
                                                         accs[:, 0:nchunks],
                                                         axis=AX.X)
                                    nc.vector.reciprocal(se[:], se[:])
                                wid = (i + 1) * 128
                                nc.vector.tensor_scalar_mul(
                                    pn[:, 0:wid], pn[:, 0:wid], se[:, 0:1])
                                nc.sync.dma_start(
                                    out=attn_w[h, i * 128:(i + 1) * 128, 0:wid],
                                    in_=pn[:, 0:wid])
                                nc.sync.dma_start(
                                    out=rrow_dr[h, i * 128:(i + 1) * 128],
                                    in_=se[:, 0:1])
                        # transposed pass
                        rb = []
                        for pr in range(2):
                            t = rbp.tile([128, 512], F32, tag="rb")
                            nc.sync.dma_start(
                                out=t[0:64, :],
                                in_=rrow_dr[2 * pr, jsl].unsqueeze(0)
                                    .partition_broadcast(64))
                            nc.sync.dma_start(
                                out=t[64:128, :],
                                in_=rrow_dr[2 * pr + 1, jsl].unsqueeze(0)
                                    .partition_broadcast(64))
                            rb.append(t)
                        av = [psav.tile([128, 512], F32, tag="av",
                                        name=f"av{j}_{p2}") for p2 in range(2)]
                        nk = 4 * (j + 1)
                        for kc in range(nk):
                            et = []
                            for h in range(NQ):
                                hb = (h % 2) * 64
                                pst = ps.tile([128, 512], F32, tag="mm")
                                nc.tensor.matmul(
                                    pst[:],
                                    kkT[hb:hb + 64, kc * 128:(kc + 1) * 128],
                                    qT[h // 2][hb:hb + 64, jsl],
                                    start=True, stop=True,
                                    tile_position=(hb, 0))
                                if kc >= 4 * j:
                                    nc.vector.tensor_tensor(pst[:], pst[:],
                                                            mT[kc - 4 * j][:],
                                                            OP.add)
                                e = expp.tile([128, 512], BF16, tag="expT")
                                nc.scalar.activation(e[:], pst[:], AF.Exp)
                                et.append(e)
                            for pr in range(2):
                                for z in range(2):
                                    nc.tensor.matmul(
                                        av[pr][z * 64:(z + 1) * 64, :],
                                        vN[:, kc * 64:(kc + 1) * 64],
                                        et[2 * pr + z][:],
                                        start=(kc == 0), stop=(kc == nk - 1),
                                        tile_position=(0, z * 64))
                        for pr in range(2):
                            nc.vector.tensor_tensor(aoT[pr][j][:],
                                                    f32r(av[pr][:]),
                                                    f32r(rb[pr][:]), OP.mult)

                # ---- o-proj + AR1 ----
                with tc.tile_pool(name="wop", bufs=1) as wop, \
                     tc.tile_pool(name="obp", bufs=4) as obp:
                    wo_t = [wop.tile([128, HID], F32R, name=f"wo{i}")
                            for i in range(2)]
                    for i in range(2):
                        nc.sync.dma_start(out=wo_t[i][:],
                                          in_=f32r(woT[i * 128:(i + 1) * 128, :]))
                    for g in range(SC):
                        for ih in range(NT):
                            pst = ps.tile([128, 512], F32, tag="mm")
                            for fc in range(2):
                                nc.tensor.matmul(
                                    pst[:], wo_t[fc][:, ih * 128:(ih + 1) * 128],
                                    aoT[fc][g][:],
                                    start=(fc == 0), stop=(fc == 1))
                            ob = obp.tile([128, 512], BF16, tag="ob")
                            nc.scalar.copy(ob[:], pst[:])
                            nc.sync.dma_start(
                                out=ar1_in[g][ih * 128:(ih + 1) * 128, :],
                                in_=ob[:])
                        if os.environ.get("KERNEL_SIM_NOCOLL"):
                            nc.sync.dma_start(out=ar1_out[g][:],
                                              in_=ar1_in[g][:])
                        else:
                            nc.gpsimd.collective_compute(
                                "AllReduce", OP.add, replica_groups=RG,
                                ins=[ar1_in[g][:].opt()],
                                outs=[ar1_out[g][:].opt()])
                        nc.sync.dma_start(
                            out=o_attn.rearrange("h (g s) -> h g s", g=SC)[:, g, :],
                            in_=ar1_out[g][:])

            # ================= MLP super-phase =================
            with tc.tile_pool(name="actp", bufs=1) as actp:
                actT = [actp.tile([128, S], BF16, name=f"act{f}")
                        for f in range(8)]

                with tc.tile_pool(name="hp", bufs=1) as hp, \
                     tc.tile_pool(name="xt2p", bufs=2) as xt2p, \
                     tc.tile_pool(name="sq2p", bufs=3) as sq2p, \
                     tc.tile_pool(name="wmlp", bufs=3) as wmp, \
                     tc.tile_pool(name="ygp", bufs=3) as ygp, \
                     tc.tile_pool(name="rowp3", bufs=4) as rowp3:
                    hsb = [[hp.tile([128, 512], BF16, name=f"h{i}_{g}")
                            for g in range(SC)] for i in range(NT)]
                    rbc2 = []
                    for g in range(SC):
                        for i in range(NT):
                            xt2 = xt2p.tile([128, 512], F32, tag="xt2")
                            nc.sync.dma_start(
                                out=xt2[:],
                                in_=xT[i * 128:(i + 1) * 128,
                                       g * 512:(g + 1) * 512])
                            at2 = xt2p.tile([128, 512], BF16, tag="at2")
                            nc.sync.dma_start(
                                out=at2[:],
                                in_=ar1_out[g][i * 128:(i + 1) * 128, :])
                            nc.vector.tensor_tensor(hsb[i][g][:], xt2[:],
                                                    at2[:], OP.add)
                    for sn in range(SC):
                        sl = slice(sn * 512, (sn + 1) * 512)
                        ssq2 = psq.tile([1, 512], F32, tag="ssq",
                                        name=f"ssq2{sn}")
                        for i in range(NT):
                            sq = sq2p.tile([128, 512], F32R, tag="sq2")
                            nc.vector.tensor_tensor(sq[:], hsb[i][sn][:],
                                                    hsb[i][sn][:], OP.mult)
                            nc.tensor.matmul(ssq2[:], f32r(ones_t[:]), sq[:],
                                             start=(i == 0), stop=(i == NT - 1))
                        srow = rowp3.tile([1, 512], F32, tag="srow2")
                        nc.scalar.activation(srow[:], ssq2[:], AF.Sqrt,
                                             bias=EPS, scale=1.0 / HID)
                        rrow = rowp3.tile([1, 512], F32R, tag="rrow2")
                        with nc.allow_low_precision(reason="f32r rstd bcast"):
                            nc.vector.reciprocal(rrow[:], srow[:])
                        rb2ps = psq.tile([128, 512], F32, tag="ssq",
                                         name=f"rbc2p{sn}")
                        nc.tensor.matmul(rb2ps[:], ones_r[:], rrow[:],
                                         start=True, stop=True)
                        rb2 = rowp3.tile([128, 512], F32, tag="rbc2",
                                         name=f"rbc2_{sn}")
                        nc.scalar.copy(rb2[:], rb2ps[:])
                        rbc2.append(rb2)

                    gsb = {}
                    for fgrp in range(8):
                        for kind in range(2):
                            wsrc = wgT if kind == 0 else wuT
                            wt = wmp.tile([128, NT * 128], BF16, tag="wmlp")
                            nc.sync.dma_start(
                                out=wt[:].rearrange("p (t f) -> p t f", t=NT),
                                in_=wsrc[:, fgrp * 128:(fgrp + 1) * 128]
                                    .rearrange("(t p) f -> p t f", p=128))
                            for sn in range(SC):
                                sl = slice(sn * 512, (sn + 1) * 512)
                                pst = ps.tile([128, 512], F32, tag="mm")
                                for t in range(NT):
                                    nc.tensor.matmul(
                                        pst[:], wt[:, t * 128:(t + 1) * 128],
                                        hsb[t][sn][:],
                                        start=(t == 0), stop=(t == NT - 1))
                                if kind == 0:
                                    yg = ygp.tile([128, 512], F32, tag="yg")
                                    nc.vector.tensor_tensor(yg[:], pst[:],
                                                            rbc2[sn][:],
                                                            OP.mult)
                                    g2 = ygp.tile([128, 512], F32, tag="g2",
                                                  name=f"g2_{fgrp}_{sn}")
                                    nc.scalar.activation(g2[:], yg[:], AF.Silu)
                                    gsb[(fgrp, sn)] = g2
                                else:
                                    yu = ygp.tile([128, 512], F32, tag="yu")
                                    nc.vector.tensor_tensor(yu[:], pst[:],
                                                            rbc2[sn][:],
                                                            OP.mult)
                                    nc.vector.tensor_tensor(
                                        actT[fgrp][:, sl], yu[:],
                                        gsb[(fgrp, sn)][:], OP.mult)

                # ---- down-proj + AR2 ----
                with tc.tile_pool(name="wdp", bufs=1) as wdp, \
                     tc.tile_pool(name="dbp", bufs=3) as dbp:
                    wd_t = [wdp.tile([128, HID], BF16, name=f"wd{f}")
                            for f in range(8)]
                    for f in range(8):
                        nc.sync.dma_start(out=wd_t[f][:],
                                          in_=wdT[f * 128:(f + 1) * 128, :])
                    for sn in range(SC):
                        for ih in range(NT):
                            pst = ps.tile([128, 512], F32, tag="mm")
                            for f in range(8):
                                nc.tensor.matmul(
                                    pst[:], wd_t[f][:, ih * 128:(ih + 1) * 128],
                                    actT[f][:, sn * 512:(sn + 1) * 512],
                                    start=(f == 0), stop=(f == 7))
                            db = dbp.tile([128, 512], BF16, tag="db")
                            nc.scalar.copy(db[:], pst[:])
                            nc.sync.dma_start(
                                out=ar2_in[sn][ih * 128:(ih + 1) * 128, :],
                                in_=db[:])
                        if os.environ.get("KERNEL_SIM_NOCOLL"):
                            nc.sync.dma_start(out=ar2_out[sn][:],
                                              in_=ar2_in[sn][:])
                        else:
                            nc.gpsimd.collective_compute(
                                "AllReduce", OP.add, replica_groups=RG,
                                ins=[ar2_in[sn][:].opt()],
                                outs=[ar2_out[sn][:].opt()])
                        nc.sync.dma_start(
                            out=o_mlp.rearrange("h (g s) -> h g s", g=SC)[:, sn, :],
                            in_=ar2_out[sn][:])
    nc.finalize()
    return nc


def kernel(hidden_states, attention_mask, position_ids, wq, wk, wv, wo,
           w_gate, w_up, w_down, ln1_w, ln2_w, **kwargs):
    import ml_dtypes
    hs = np.asarray(hidden_states, dtype=np.float32)
    x = hs[0]                                   # [S, HID]
    xT = np.ascontiguousarray(x.T)
    am = np.asarray(attention_mask, dtype=np.float32)[0, 0]
    pos = np.asarray(position_ids)[0].astype(np.int64)

    inv_freq = 1.0 / (ROPE_THETA ** (np.arange(0, D, 2, dtype=np.float32) / D))
    freqs = np.outer(pos.astype(np.float32), inv_freq)
    emb = np.concatenate([freqs, freqs], axis=-1)
    cosT = np.cos(emb).T.astype(np.float32)
    sinT = np.sin(emb).T.astype(np.float32)
    sinsT = sinT.copy()
    sinsT[0:32] = -sinT[0:32]
    cos2 = np.ascontiguousarray(np.tile(cosT, (2, 1)))
    sins2 = np.ascontiguousarray(np.tile(sinsT, (2, 1)))

    maskN = np.ascontiguousarray(am[0:512, 0:512])
    maskTm = np.ascontiguousarray(am[0:512, 0:512].T)
    ones_hv = np.ones((128, 1), dtype=np.float32)

    ln1 = np.asarray(ln1_w, dtype=np.float32)
    ln2 = np.asarray(ln2_w, dtype=np.float32)
    wq = np.asarray(wq, dtype=np.float32)
    wk = np.asarray(wk, dtype=np.float32)
    wv = np.asarray(wv, dtype=np.float32)
    wo = np.asarray(wo, dtype=np.float32)
    w_gate = np.asarray(w_gate, dtype=np.float32)
    w_up = np.asarray(w_up, dtype=np.float32)
    w_down = np.asarray(w_down, dtype=np.float32)

    in_maps = []
    for c in range(NCORES):
        wq_s = (wq[c * FQ:(c + 1) * FQ, :] * ln1[None, :]) / np.sqrt(D)
        wk_s = wk[c * D:(c + 1) * D, :] * ln1[None, :]
        wv_s = wv[c * D:(c + 1) * D, :] * ln1[None, :]
        wkv_s = np.concatenate([wk_s, wv_s], axis=0)
        wo_s = wo[:, c * FQ:(c + 1) * FQ]
        wg_s = w_gate[c * INT_SH:(c + 1) * INT_SH, :] * ln2[None, :]
        wu_s = w_up[c * INT_SH:(c + 1) * INT_SH, :] * ln2[None, :]
        wd_s = w_down[:, c * INT_SH:(c + 1) * INT_SH]
        in_maps.append(dict(
            xT=xT,
            wqT=np.ascontiguousarray(wq_s.T.astype(np.float32)),
            wkvT=np.ascontiguousarray(wkv_s.T.astype(np.float32)),
            woT=np.ascontiguousarray(wo_s.T.astype(np.float32)),
            wgT=np.ascontiguousarray(wg_s.T).astype(ml_dtypes.bfloat16),
            wuT=np.ascontiguousarray(wu_s.T).astype(ml_dtypes.bfloat16),
            wdT=np.ascontiguousarray(wd_s.T).astype(ml_dtypes.bfloat16),
            cos2=cos2, sins2=sins2, maskN=maskN, maskT=maskTm,
            ones_in=ones_hv, ones_row=ones_hv[:, 0:1].T.copy(),
        ))

    if 'nc' not in _CACHED:
        _CACHED['nc'] = _build()
    res = run_bass_kernel_spmd(_CACHED['nc'], in_maps,
                               core_ids=list(range(NCORES)), **kwargs)
    _CACHED['last_results'] = res

    r0 = res.results[0]
    attn_sum = np.asarray(r0["o_attn"]).astype(np.float32).T
    mlp_sum = np.asarray(r0["o_mlp"]).astype(np.float32).T
    out = x + attn_sum + mlp_sum
    aw = np.concatenate([np.asarray(res.results[c]["attn_w"])[None]
                         for c in range(NCORES)], axis=1)
    return (out[None].astype(np.float32), aw.astype(np.float32))


# revision 22
# speedup vs baseline: 1.0080x; 1.0080x over previous
"""Llama decoder layer on 8 TRN2 NeuronCores, tensor-parallel over heads.

Core c owns q-heads 4c..4c+3 (one GQA group -> kv head c); wq/wk/wv and
gate/up are column-sharded, wo/down row-sharded; partial sums cross cores via
two bf16 AllReduces (chunked by hid for pipelining).  On-device activations
live transposed ([feature, seq]); the host transposes inputs/outputs and folds
ln weights + 1/sqrt(D) into the projections.  The RMSNorm rsqrt scale is
per-sequence-position, so it commutes past the hid-contraction: raw x feeds
the matmuls and the scale fuses into the PSUM->SBUF copies.  Attention path is
float32r (full PE rate, ~fp32 precision); MLP runs bf16.
"""
import sys, os
sys.path.insert(0, '/opt/trn_rl_repo')
import numpy as np
import concourse.bacc as bacc
import concourse.mybir as mybir
import concourse.tile as tile
from concourse.bass_utils import run_bass_kernel_spmd
from concourse.masks import make_identity

F32 = mybir.dt.float32
F32R = mybir.dt.float32r
BF16 = mybir.dt.bfloat16
AF = mybir.ActivationFunctionType
OP = mybir.AluOpType
AX = mybir.AxisListType

NCORES = 8
S = 2048
HID = 2048
D = 64
NQ = 4            # q heads per core
FQ = NQ * D       # 256 q features per core
INT_SH = 1024     # intermediate shard per core
NT = HID // 128   # 16 hid tiles
ST = S // 128     # 16 seq tiles
SC = S // 512     # 4 seq chunks
EPS = 1e-6
ROPE_THETA = 10000.0

_CACHED = {}


def _build():
    nc = bacc.Bacc("TRN2", target_bir_lowering=False, num_devices=NCORES)
    _eps_t = nc.alloc_sbuf_tensor("const-eps", [128, 1], F32)
    nc.gpsimd.memset(_eps_t.ap(), EPS)
    nc.const_aps.aps[(F32, EPS)] = _eps_t.ap()
    nc.all_engine_barrier()

    xT = nc.dram_tensor("xT", [HID, S], F32, kind="ExternalInput")
    wqT = nc.dram_tensor("wqT", [HID, FQ], F32, kind="ExternalInput")
    wkvT = nc.dram_tensor("wkvT", [HID, 128], F32, kind="ExternalInput")
    woT = nc.dram_tensor("woT", [FQ, HID], F32, kind="ExternalInput")
    wgT = nc.dram_tensor("wgT", [HID, INT_SH], BF16, kind="ExternalInput")
    wuT = nc.dram_tensor("wuT", [HID, INT_SH], BF16, kind="ExternalInput")
    wdT = nc.dram_tensor("wdT", [INT_SH, HID], BF16, kind="ExternalInput")
    cos2 = nc.dram_tensor("cos2", [128, S], F32, kind="ExternalInput")
    sins2 = nc.dram_tensor("sins2", [128, S], F32, kind="ExternalInput")
    maskN = nc.dram_tensor("maskN", [512, 512], F32, kind="ExternalInput")
    maskT = nc.dram_tensor("maskT", [512, 512], F32, kind="ExternalInput")
    ones_in = nc.dram_tensor("ones_in", [128, 1], F32, kind="ExternalInput")
    ones_row_in = nc.dram_tensor("ones_row", [1, 128], F32, kind="ExternalInput")

    attn_w = nc.dram_tensor("attn_w", [NQ, S, S], F32, kind="ExternalOutput")
    o_attn = nc.dram_tensor("o_attn", [HID, S], BF16, kind="ExternalOutput")
    o_mlp = nc.dram_tensor("o_mlp", [HID, S], BF16, kind="ExternalOutput")

    def f32r(ap):
        return ap.bitcast(F32R)

    RG = [list(range(NCORES))]

    with tile.TileContext(nc) as tc:
        with tc.tile_pool(name="dram", bufs=1, space="DRAM") as dr, \
             tc.tile_pool(name="cpool", bufs=1) as cpool, \
             tc.tile_pool(name="psmm", bufs=5, space="PSUM") as ps, \
             tc.tile_pool(name="psav", bufs=2, space="PSUM") as psav, \
             tc.tile_pool(name="psq", bufs=1, space="PSUM") as psq:

            ar1_in = [dr.tile([HID, 512], BF16, name=f"ar1i{g}")
                      for g in range(4)]
            ar1_out = [dr.tile([HID, 512], BF16, addr_space="Shared",
                               name=f"ar1o{g}") for g in range(4)]
            ar2_in = [dr.tile([HID, 512], BF16, name=f"ar2i{g}")
                      for g in range(4)]
            ar2_out = [dr.tile([HID, 512], BF16, addr_space="Shared",
                               name=f"ar2o{g}") for g in range(4)]
            rrow_dr = dr.tile([NQ, S], F32)

            ones_t = cpool.tile([128, 1], F32R, name="ones_t")
            nc.sync.dma_start(out=ones_t[:], in_=f32r(ones_in[:]))
            ones_r = cpool.tile([1, 128], F32R, name="ones_r")
            nc.sync.dma_start(out=ones_r[:], in_=f32r(ones_row_in[:]))
            ident = cpool.tile([128, 128], F32, name="ident")
            make_identity(nc, ident[:])

            # ================= attention super-phase =================
            with tc.tile_pool(name="attp", bufs=1) as ap_:
                qT = [ap_.tile([128, S], F32R, name=f"qT{i}") for i in range(2)]
                kvT = ap_.tile([128, S], F32R, name="kvT")
                kkT = ap_.tile([128, S], F32R, name="kkT")
                vN = ap_.tile([128, ST * D], BF16, name="vN")
                aoT = [[ap_.tile([128, 512], F32R, name=f"aoT{i}_{j}")
                        for j in range(SC)] for i in range(2)]

                # ---- norm1 stats + QKV ----
                with tc.tile_pool(name="xsp", bufs=20) as xsp, \
                     tc.tile_pool(name="sqp", bufs=3) as sqp, \
                     tc.tile_pool(name="wqp", bufs=1) as wqp, \
                     tc.tile_pool(name="ropep", bufs=1) as rp, \
                     tc.tile_pool(name="rotp", bufs=3) as rotp, \
                     tc.tile_pool(name="rowp", bufs=4) as rowp:
                    cos_t = rp.tile([128, S], F32, name="cos_t")
                    nc.sync.dma_start(out=cos_t[:], in_=cos2[:])
                    sin_t = rp.tile([128, S], F32, name="sin_t")
                    nc.sync.dma_start(out=sin_t[:], in_=sins2[:])
                    wq_t = [wqp.tile([128, FQ], F32R, name=f"wq{i}")
                            for i in range(NT)]
                    wkv_t = [wqp.tile([128, 128], F32R, name=f"wkv{i}")
                             for i in range(NT)]
                    for sn in range(SC):
                        sl = slice(sn * 512, (sn + 1) * 512)
                        ssq = psq.tile([1, 512], F32, tag="ssq", name=f"ssq{sn}")
                        xrow = []
                        for i in range(NT):
                            xt = xsp.tile([128, 512], F32R, name=f"x_{sn}_{i}",
                                          tag="xs")
                            nc.sync.dma_start(
                                out=xt[:],
                                in_=f32r(xT[i * 128:(i + 1) * 128,
                                            sn * 512:(sn + 1) * 512]))
                            xrow.append(xt)
                            sq = sqp.tile([128, 512], F32R, tag="sq")
                            nc.vector.tensor_tensor(sq[:], xt[:], xt[:], OP.mult)
                            nc.tensor.matmul(ssq[:], f32r(ones_t[:]), sq[:],
                                             start=(i == 0), stop=(i == NT - 1))
                        if sn == 0:
                            for i in range(NT):
                                nc.sync.dma_start(
                                    out=wq_t[i][:],
                                    in_=f32r(wqT[i * 128:(i + 1) * 128, :]))
                                nc.sync.dma_start(
                                    out=wkv_t[i][:],
                                    in_=f32r(wkvT[i * 128:(i + 1) * 128, :]))
                        srow = rowp.tile([1, 512], F32, tag="srow")
                        nc.scalar.activation(srow[:], ssq[:], AF.Sqrt,
                                             bias=EPS, scale=1.0 / HID)
                        rrow = rowp.tile([1, 512], F32R, tag="rrow")
                        with nc.allow_low_precision(reason="f32r rstd bcast"):
                            nc.vector.reciprocal(rrow[:], srow[:])
                        rbp_ps = psq.tile([128, 512], F32, tag="ssq",
                                          name=f"rbcp{sn}")
                        nc.tensor.matmul(rbp_ps[:], ones_r[:], rrow[:],
                                         start=True, stop=True)
                        rbc = rowp.tile([128, 512], F32, tag="rbc",
                                        name=f"rbc{sn}")
                        nc.scalar.copy(rbc[:], rbp_ps[:])
                        for fc in range(2):
                            pst = ps.tile([128, 512], F32, tag="mm")
                            for i in range(NT):
                                nc.tensor.matmul(
                                    pst[:], wq_t[i][:, fc * 128:(fc + 1) * 128],
                                    xrow[i][:],
                                    start=(i == 0), stop=(i == NT - 1))
                            nc.vector.tensor_tensor(qT[fc][:, sl], f32r(pst[:]),
                                                    f32r(rbc[:]), OP.mult)
                        pst = ps.tile([128, 512], F32, tag="mm")
                        for i in range(NT):
                            nc.tensor.matmul(pst[:], wkv_t[i][:], xrow[i][:],
                                             start=(i == 0), stop=(i == NT - 1))
                        nc.vector.tensor_tensor(kvT[:, sl], f32r(pst[:]),
                                                f32r(rbc[:]), OP.mult)
                        # rope on this s-chunk
                        for tq in range(3):
                            tgt = qT[tq] if tq < 2 else kvT
                            hi = 128 if tq < 2 else 64
                            rot = rotp.tile([128, 512], F32R, tag="rot")
                            for b in range(0, hi, 64):
                                nc.vector.tensor_copy(
                                    rot[b:b + 32, :], tgt[b + 32:b + 64, sl])
                                nc.vector.tensor_copy(
                                    rot[b + 32:b + 64, :], tgt[b:b + 32, sl])
                            nc.vector.tensor_tensor(tgt[0:hi, sl], tgt[0:hi, sl],
                                                    f32r(cos_t[0:hi, sl]),
                                                    OP.mult)
                            nc.vector.tensor_tensor(rot[0:hi, :], rot[0:hi, :],
                                                    f32r(sin_t[0:hi, sl]),
                                                    OP.mult)
                            nc.vector.tensor_tensor(tgt[0:hi, sl], tgt[0:hi, sl],
                                                    rot[0:hi, :], OP.add)
                        nc.vector.tensor_copy(kkT[0:64, sl], kvT[0:64, sl])
                        nc.vector.tensor_copy(kkT[64:128, sl], kvT[0:64, sl])
                        vtmp = rotp.tile([64, 512], F32R, tag="vtmp")
                        nc.vector.tensor_copy(vtmp[:], kvT[64:128, sl])
                        for stl in range(4):
                            st = sn * 4 + stl
                            pst = ps.tile([128, 64], F32, tag="mm")
                            nc.tensor.transpose(
                                pst[:],
                                vtmp[:, stl * 128:(stl + 1) * 128].bitcast(F32),
                                ident[0:64, 0:64])
                            nc.scalar.copy(vN[:, st * 64:(st + 1) * 64], pst[:])

                # ---- attention ----
                with tc.tile_pool(name="mskp", bufs=1) as mp_, \
                     tc.tile_pool(name="pnp", bufs=4) as pnp, \
                     tc.tile_pool(name="expp", bufs=10) as expp, \
                     tc.tile_pool(name="rbp", bufs=3) as rbp, \
                     tc.tile_pool(name="rowp2", bufs=4) as rowp2:
                    mN = []
                    mT = []
                    for m in range(4):
                        t1 = mp_.tile([128, 512], F32, name=f"mN{m}")
                        nc.sync.dma_start(out=t1[:],
                                          in_=maskN[m * 128:(m + 1) * 128, :])
                        mN.append(t1)
                        t2 = mp_.tile([128, 512], F32, name=f"mT{m}")
                        nc.sync.dma_start(out=t2[:],
                                          in_=maskT[m * 128:(m + 1) * 128, :])
                        mT.append(t2)

                    for j in range(SC):
                        jsl = slice(j * 512, (j + 1) * 512)
                        # normal pass: attn_w rows + 1/sumexp
                        for i in range(4 * j, 4 * j + 4):
                            nchunks = i // 4 + 1
                            for h in range(NQ):
                                p = h // 2
                                hb = (h % 2) * 64
                                pn = pnp.tile([128, S], F32, tag="pn")
                                accs = rowp2.tile([128, 4], F32, tag="accs")
                                for n in range(nchunks):
                                    w = min(512, (i + 1) * 128 - n * 512)
                                    pst = ps.tile([128, 512], F32, tag="mm")
                                    nc.tensor.matmul(
                                        pst[:, 0:w],
                                        qT[p][hb:hb + 64, i * 128:(i + 1) * 128],
                                        kkT[hb:hb + 64, n * 512:n * 512 + w],
                                        start=True, stop=True,
                                        tile_position=(hb, 0))
                                    if n == i // 4:
                                        m0 = (i % 4) * 128
                                        nc.vector.tensor_tensor(
                                            pst[:, m0:w], pst[:, m0:w],
                                            mN[i % 4][:, m0:w], OP.add)
                                    nc.scalar.activation(
                                        pn[:, n * 512:n * 512 + w],
                                        pst[:, 0:w], AF.Exp,
                                        accum_out=accs[:, n:n + 1])
                                se = rowp2.tile([128, 1], F32, tag="se")
                                if nchunks == 1:
                                    nc.vector.reciprocal(se[:], accs[:, 0:1])
                                else:
                                    nc.vector.reduce_sum(se[:],
                                                         accs[:, 0:nchunks],
                                                         axis=AX.X)
                                    nc.vector.reciprocal(se[:], se[:])
                                wid = (i + 1) * 128
                                nc.vector.tensor_scalar_mul(
                                    pn[:, 0:wid], pn[:, 0:wid], se[:, 0:1])
                                nc.sync.dma_start(
                                    out=attn_w[h, i * 128:(i + 1) * 128, 0:wid],
                                    in_=pn[:, 0:wid])
                                nc.sync.dma_start(
                                    out=rrow_dr[h, i * 128:(i + 1) * 128],
                                    in_=se[:, 0:1])
                        # transposed pass
                        rb = []
                        for pr in range(2):
                            t = rbp.tile([128, 512], F32, tag="rb")
                            nc.sync.dma_start(
                                out=t[0:64, :],
                                in_=rrow_dr[2 * pr, jsl].unsqueeze(0)
                                    .partition_broadcast(64))
                            nc.sync.dma_start(
                                out=t[64:128, :],
                                in_=rrow_dr[2 * pr + 1, jsl].unsqueeze(0)
                                    .partition_broadcast(64))
                            rb.append(t)
                        av = [psav.tile([128, 512], F32, tag="av",
                                        name=f"av{j}_{p2}") for p2 in range(2)]
                        nk = 4 * (j + 1)
                        for kc in range(nk):
                            et = []
                            for h in range(NQ):
                                hb = (h % 2) * 64
                                pst = ps.tile([128, 512], F32, tag="mm")
                                nc.tensor.matmul(
                                    pst[:],
                                    kkT[hb:hb + 64, kc * 128:(kc + 1) * 128],
                                    qT[h // 2][hb:hb + 64, jsl],
                                    start=True, stop=True,
                                    tile_position=(hb, 0))
                                if kc >= 4 * j:
                                    nc.vector.tensor_tensor(pst[:], pst[:],
                                                            mT[kc - 4 * j][:],
                                                            OP.add)
                                e = expp.tile([128, 512], BF16, tag="expT")
                                nc.scalar.activation(e[:], pst[:], AF.Exp)
                                et.append(e)
                            for pr in range(2):
                                for z in range(2):
                                    nc.tensor.matmul(
                                        av[pr][z * 64:(z + 1) * 64, :],
                                        vN[:, kc * 64:(kc + 1) * 64],
                                        et[2 * pr + z][:],
                                        start=(kc == 0), stop=(kc == nk - 1),
                                        tile_position=(0, z * 64))
                        for pr in range(2):
                            nc.vector.tensor_tensor(aoT[pr][j][:],
                                                    f32r(av[pr][:]),
                                                    f32r(rb[pr][:]), OP.mult)

                # ---- o-proj + AR1 ----
                with tc.tile_pool(name="wop", bufs=1) as wop, \
                     tc.tile_pool(name="obp", bufs=4) as obp:
                    wo_t = [wop.tile([128, HID], F32R, name=f"wo{i}")
                            for i in range(2)]
                    for i in range(2):
                        nc.sync.dma_start(out=wo_t[i][:],
                                          in_=f32r(woT[i * 128:(i + 1) * 128, :]))
                    for g in range(SC):
                        for ih in range(NT):
                            pst = ps.tile([128, 512], F32, tag="mm")
                            for fc in range(2):
                                nc.tensor.matmul(
                                    pst[:], wo_t[fc][:, ih * 128:(ih + 1) * 128],
                                    aoT[fc][g][:],
                                    start=(fc == 0), stop=(fc == 1))
                            ob = obp.tile([128, 512], BF16, tag="ob")
                            nc.scalar.copy(ob[:], pst[:])
                            nc.sync.dma_start(
                                out=ar1_in[g][ih * 128:(ih + 1) * 128, :],
                                in_=ob[:])
                        if os.environ.get("KERNEL_SIM_NOCOLL"):
                            nc.sync.dma_start(out=ar1_out[g][:],
                                              in_=ar1_in[g][:])
                        else:
                            nc.gpsimd.collective_compute(
                                "AllReduce", OP.add, replica_groups=RG,
                                ins=[ar1_in[g][:].opt()],
                                outs=[ar1_out[g][:].opt()])
                        nc.sync.dma_start(
                            out=o_attn.rearrange("h (g s) -> h g s", g=SC)[:, g, :],
                            in_=ar1_out[g][:])

            # ================= MLP super-phase =================
            with tc.tile_pool(name="actp", bufs=1) as actp:
                actT = [actp.tile([128, S], BF16, name=f"act{f}")
                        for f in range(8)]

                with tc.tile_pool(name="hp", bufs=1) as hp, \
                     tc.tile_pool(name="xt2p", bufs=2) as xt2p, \
                     tc.tile_pool(name="sq2p", bufs=3) as sq2p, \
                     tc.tile_pool(name="wmlp", bufs=3) as wmp, \
                     tc.tile_pool(name="ygp", bufs=3) as ygp, \
                     tc.tile_pool(name="rowp3", bufs=4) as rowp3:
                    hsb = [[hp.tile([128, 512], BF16, name=f"h{i}_{g}")
                            for g in range(SC)] for i in range(NT)]
                    rbc2 = []
                    for g in range(SC):
                        for i in range(NT):
                            xt2 = xt2p.tile([128, 512], F32, tag="xt2")
                            nc.sync.dma_start(
                                out=xt2[:],
                                in_=xT[i * 128:(i + 1) * 128,
                                       g * 512:(g + 1) * 512])
                            at2 = xt2p.tile([128, 512], BF16, tag="at2")
                            nc.sync.dma_start(
                                out=at2[:],
                                in_=ar1_out[g][i * 128:(i + 1) * 128, :])
                            nc.vector.tensor_tensor(hsb[i][g][:], xt2[:],
                                                    at2[:], OP.add)
                    for sn in range(SC):
                        sl = slice(sn * 512, (sn + 1) * 512)
                        ssq2 = psq.tile([1, 512], F32, tag="ssq",
                                        name=f"ssq2{sn}")
                        for i in range(NT):
                            sq = sq2p.tile([128, 512], F32R, tag="sq2")
                            nc.vector.tensor_tensor(sq[:], hsb[i][sn][:],
                                                    hsb[i][sn][:], OP.mult)
                            nc.tensor.matmul(ssq2[:], f32r(ones_t[:]), sq[:],
                                             start=(i == 0), stop=(i == NT - 1))
                        srow = rowp3.tile([1, 512], F32, tag="srow2")
                        nc.scalar.activation(srow[:], ssq2[:], AF.Sqrt,
                                             bias=EPS, scale=1.0 / HID)
                        rrow = rowp3.tile([1, 512], F32R, tag="rrow2")
                        with nc.allow_low_precision(reason="f32r rstd bcast"):
                            nc.vector.reciprocal(rrow[:], srow[:])
                        rb2ps = psq.tile([128, 512], F32, tag="ssq",
                                         name=f"rbc2p{sn}")
                        nc.tensor.matmul(rb2ps[:], ones_r[:], rrow[:],
                                         start=True, stop=True)
                        rb2 = rowp3.tile([128, 512], F32, tag="rbc2",
                                         name=f"rbc2_{sn}")
                        nc.scalar.copy(rb2[:], rb2ps[:])
                        rbc2.append(rb2)

                    gsb = {}
                    for fgrp in range(8):
                        for kind in range(2):
                            wsrc = wgT if kind == 0 else wuT
                            wt = wmp.tile([128, NT * 128], BF16, tag="wmlp")
                            nc.sync.dma_start(
                                out=wt[:].rearrange("p (t f) -> p t f", t=NT),
                                in_=wsrc[:, fgrp * 128:(fgrp + 1) * 128]
                                    .rearrange("(t p) f -> p t f", p=128))
                            for sn in range(SC):
                                sl = slice(sn * 512, (sn + 1) * 512)
                                pst = ps.tile([128, 512], F32, tag="mm")
                                for t in range(NT):
                                    nc.tensor.matmul(
                                        pst[:], wt[:, t * 128:(t + 1) * 128],
                                        hsb[t][sn][:],
                                        start=(t == 0), stop=(t == NT - 1))
                                if kind == 0:
                                    yg = ygp.tile([128, 512], F32, tag="yg")
                                    nc.vector.tensor_tensor(yg[:], pst[:],
                                                            rbc2[sn][:],
                                                            OP.mult)
                                    g2 = ygp.tile([128, 512], F32, tag="g2",
                                                  name=f"g2_{fgrp}_{sn}")
                                    nc.scalar.activation(g2[:], yg[:], AF.Silu)
                                    gsb[(fgrp, sn)] = g2
                                else:
                                    yu = ygp.tile([128, 512], F32, tag="yu")
                                    nc.vector.tensor_tensor(yu[:], pst[:],
                                                            rbc2[sn][:],
                                                            OP.mult)
                                    nc.vector.tensor_tensor(
                                        actT[fgrp][:, sl], yu[:],
                                        gsb[(fgrp, sn)][:], OP.mult)

                # ---- down-proj + AR2 ----
                with tc.tile_pool(name="wdp", bufs=1) as wdp, \
                     tc.tile_pool(name="dbp", bufs=3) as dbp:
                    wd_t = [wdp.tile([128, HID], BF16, name=f"wd{f}")
                            for f in range(8)]
                    for f in range(8):
                        nc.sync.dma_start(out=wd_t[f][:],
                                          in_=wdT[f * 128:(f + 1) * 128, :])
                    for sn in range(SC):
                        for ih in range(NT):
                            pst = ps.tile([128, 512], F32, tag="mm")
                            for f in range(8):
                                nc.tensor.matmul(
                                    pst[:], wd_t[f][:, ih * 128:(ih + 1) * 128],
                                    actT[f][:, sn * 512:(sn + 1) * 512],
                                    start=(f == 0), stop=(f == 7))
                            db = dbp.tile([128, 512], BF16, tag="db")
                            nc.scalar.copy(db[:], pst[:])
                            nc.sync.dma_start(
                                out=ar2_in[sn][ih * 128:(ih + 1) * 128, :],
                                in_=db[:])
                        if os.environ.get("KERNEL_SIM_NOCOLL"):
                            nc.sync.dma_start(out=ar2_out[sn][:],
                                              in_=ar2_in[sn][:])
                        else:
                            nc.gpsimd.collective_compute(
                                "AllReduce", OP.add, replica_groups=RG,
                                ins=[ar2_in[sn][:].opt()],
                                outs=[ar2_out[sn][:].opt()])
                        nc.sync.dma_start(
                            out=o_mlp.rearrange("h (g s) -> h g s", g=SC)[:, sn, :],
                            in_=ar2_out[sn][:])
    nc.finalize()
    return nc


def kernel(hidden_states, attention_mask, position_ids, wq, wk, wv, wo,
           w_gate, w_up, w_down, ln1_w, ln2_w, **kwargs):
    import ml_dtypes
    hs = np.asarray(hidden_states, dtype=np.float32)
    x = hs[0]                                   # [S, HID]
    xT = np.ascontiguousarray(x.T)
    am = np.asarray(attention_mask, dtype=np.float32)[0, 0]
    pos = np.asarray(position_ids)[0].astype(np.int64)

    inv_freq = 1.0 / (ROPE_THETA ** (np.arange(0, D, 2, dtype=np.float32) / D))
    freqs = np.outer(pos.astype(np.float32), inv_freq)
    emb = np.concatenate([freqs, freqs], axis=-1)
    cosT = np.cos(emb).T.astype(np.float32)
    sinT = np.sin(emb).T.astype(np.float32)
    sinsT = sinT.copy()
    sinsT[0:32] = -sinT[0:32]
    cos2 = np.ascontiguousarray(np.tile(cosT, (2, 1)))
    sins2 = np.ascontiguousarray(np.tile(sinsT, (2, 1)))

    maskN = np.ascontiguousarray(am[0:512, 0:512])
    maskTm = np.ascontiguousarray(am[0:512, 0:512].T)
    ones_hv = np.ones((128, 1), dtype=np.float32)

    ln1 = np.asarray(ln1_w, dtype=np.float32)
    ln2 = np.asarray(ln2_w, dtype=np.float32)
    wq = np.asarray(wq, dtype=np.float32)
    wk = np.asarray(wk, dtype=np.float32)
    wv = np.asarray(wv, dtype=np.float32)
    wo = np.asarray(wo, dtype=np.float32)
    w_gate = np.asarray(w_gate, dtype=np.float32)
    w_up = np.asarray(w_up, dtype=np.float32)
    w_down = np.asarray(w_down, dtype=np.float32)

    in_maps = []
    for c in range(NCORES):
        wq_s = (wq[c * FQ:(c + 1) * FQ, :] * ln1[None, :]) / np.sqrt(D)
        wk_s = wk[c * D:(c + 1) * D, :] * ln1[None, :]
        wv_s = wv[c * D:(c + 1) * D, :] * ln1[None, :]
        wkv_s = np.concatenate([wk_s, wv_s], axis=0)
        wo_s = wo[:, c * FQ:(c + 1) * FQ]
        wg_s = w_gate[c * INT_SH:(c + 1) * INT_SH, :] * ln2[None, :]
        wu_s = w_up[c * INT_SH:(c + 1) * INT_SH, :] * ln2[None, :]
        wd_s = w_down[:, c * INT_SH:(c + 1) * INT_SH]
        in_maps.append(dict(
            xT=xT,
            wqT=np.ascontiguousarray(wq_s.T.astype(np.float32)),
            wkvT=np.ascontiguousarray(wkv_s.T.astype(np.float32)),
            woT=np.ascontiguousarray(wo_s.T.astype(np.float32)),
            wgT=np.ascontiguousarray(wg_s.T).astype(ml_dtypes.bfloat16),
            wuT=np.ascontiguousarray(wu_s.T).astype(ml_dtypes.bfloat16),
            wdT=np.ascontiguousarray(wd_s.T).astype(ml_dtypes.bfloat16),
            cos2=cos2, sins2=sins2, maskN=maskN, maskT=maskTm,
            ones_in=ones_hv, ones_row=ones_hv[:, 0:1].T.copy(),
        ))

    if 'nc' not in _CACHED:
        _CACHED['nc'] = _build()
    res = run_bass_kernel_spmd(_CACHED['nc'], in_maps,
                               core_ids=list(range(NCORES)), **kwargs)
    _CACHED['last_results'] = res

    r0 = res.results[0]
    attn_sum = np.asarray(r0["o_attn"]).astype(np.float32).T
    mlp_sum = np.asarray(r0["o_mlp"]).astype(np.float32).T
    out = x + attn_sum + mlp_sum
    aw = np.concatenate([np.asarray(res.results[c]["attn_w"])[None]
                         for c in range(NCORES)], axis=1)
    return (out[None].astype(np.float32), aw.astype(np.float32))


# revision 26
# speedup vs baseline: 1.0093x; 1.0013x over previous
"""Llama decoder layer on 8 TRN2 NeuronCores, tensor-parallel over heads.

Core c owns q-heads 4c..4c+3 (one GQA group -> kv head c); wq/wk/wv and
gate/up are column-sharded, wo/down row-sharded; partial sums cross cores via
two bf16 AllReduces (chunked by hid for pipelining).  On-device activations
live transposed ([feature, seq]); the host transposes inputs/outputs and folds
ln weights + 1/sqrt(D) into the projections.  The RMSNorm rsqrt scale is
per-sequence-position, so it commutes past the hid-contraction: raw x feeds
the matmuls and the scale fuses into the PSUM->SBUF copies.  Attention path is
float32r (full PE rate, ~fp32 precision); MLP runs bf16.
"""
import sys, os
sys.path.insert(0, '/opt/trn_rl_repo')
import numpy as np
import concourse.bacc as bacc
import concourse.mybir as mybir
import concourse.tile as tile
from concourse.bass_utils import run_bass_kernel_spmd
from concourse.masks import make_identity

F32 = mybir.dt.float32
F32R = mybir.dt.float32r
BF16 = mybir.dt.bfloat16
AF = mybir.ActivationFunctionType
OP = mybir.AluOpType
AX = mybir.AxisListType

NCORES = 8
S = 2048
HID = 2048
D = 64
NQ = 4            # q heads per core
FQ = NQ * D       # 256 q features per core
INT_SH = 1024     # intermediate shard per core
NT = HID // 128   # 16 hid tiles
ST = S // 128     # 16 seq tiles
SC = S // 512     # 4 seq chunks
EPS = 1e-6
ROPE_THETA = 10000.0

_CACHED = {}


def _build():
    nc = bacc.Bacc("TRN2", target_bir_lowering=False, num_devices=NCORES)
    _eps_t = nc.alloc_sbuf_tensor("const-eps", [128, 1], F32)
    nc.gpsimd.memset(_eps_t.ap(), EPS)
    nc.const_aps.aps[(F32, EPS)] = _eps_t.ap()
    nc.all_engine_barrier()

    xT = nc.dram_tensor("xT", [HID, S], F32, kind="ExternalInput")
    wqT = nc.dram_tensor("wqT", [HID, FQ], F32, kind="ExternalInput")
    wkvT = nc.dram_tensor("wkvT", [HID, 128], F32, kind="ExternalInput")
    woT = nc.dram_tensor("woT", [FQ, HID], F32, kind="ExternalInput")
    wgT = nc.dram_tensor("wgT", [HID, INT_SH], BF16, kind="ExternalInput")
    wuT = nc.dram_tensor("wuT", [HID, INT_SH], BF16, kind="ExternalInput")
    wdT = nc.dram_tensor("wdT", [INT_SH, HID], BF16, kind="ExternalInput")
    cos2 = nc.dram_tensor("cos2", [128, S], F32, kind="ExternalInput")
    sins2 = nc.dram_tensor("sins2", [128, S], F32, kind="ExternalInput")
    maskN = nc.dram_tensor("maskN", [512, 512], F32, kind="ExternalInput")
    maskT = nc.dram_tensor("maskT", [512, 512], F32, kind="ExternalInput")
    ones_in = nc.dram_tensor("ones_in", [128, 1], F32, kind="ExternalInput")
    ones_row_in = nc.dram_tensor("ones_row", [1, 128], F32, kind="ExternalInput")

    attn_w = nc.dram_tensor("attn_w", [NQ, S, S], BF16, kind="ExternalOutput")
    o_attn = nc.dram_tensor("o_attn", [HID, S], BF16, kind="ExternalOutput")
    o_mlp = nc.dram_tensor("o_mlp", [HID, S], BF16, kind="ExternalOutput")

    def f32r(ap):
        return ap.bitcast(F32R)

    RG = [list(range(NCORES))]

    with tile.TileContext(nc) as tc:
        with tc.tile_pool(name="dram", bufs=1, space="DRAM") as dr, \
             tc.tile_pool(name="cpool", bufs=1) as cpool, \
             tc.tile_pool(name="psmm", bufs=5, space="PSUM") as ps, \
             tc.tile_pool(name="psav", bufs=2, space="PSUM") as psav, \
             tc.tile_pool(name="psq", bufs=1, space="PSUM") as psq:

            ar1_in = [dr.tile([HID, 512], BF16, name=f"ar1i{g}")
                      for g in range(4)]
            ar1_out = [dr.tile([HID, 512], BF16, addr_space="Shared",
                               name=f"ar1o{g}") for g in range(4)]
            ar2_in = [dr.tile([HID, 512], BF16, name=f"ar2i{g}")
                      for g in range(4)]
            ar2_out = [dr.tile([HID, 512], BF16, addr_space="Shared",
                               name=f"ar2o{g}") for g in range(4)]
            rrow_dr = dr.tile([NQ, S], F32)

            ones_t = cpool.tile([128, 1], F32R, name="ones_t")
            nc.sync.dma_start(out=ones_t[:], in_=f32r(ones_in[:]))
            ones_r = cpool.tile([1, 128], F32R, name="ones_r")
            nc.sync.dma_start(out=ones_r[:], in_=f32r(ones_row_in[:]))
            ident = cpool.tile([128, 128], F32, name="ident")
            make_identity(nc, ident[:])

            # ================= attention super-phase =================
            with tc.tile_pool(name="attp", bufs=1) as ap_:
                qT = [ap_.tile([128, S], F32R, name=f"qT{i}") for i in range(2)]
                kvT = ap_.tile([128, S], F32R, name="kvT")
                kkT = ap_.tile([128, S], F32R, name="kkT")
                vN = ap_.tile([128, ST * D], BF16, name="vN")
                aoT = [[ap_.tile([128, 512], F32R, name=f"aoT{i}_{j}")
                        for j in range(SC)] for i in range(2)]

                # ---- norm1 stats + QKV ----
                with tc.tile_pool(name="xsp", bufs=20) as xsp, \
                     tc.tile_pool(name="sqp", bufs=3) as sqp, \
                     tc.tile_pool(name="wqp", bufs=1) as wqp, \
                     tc.tile_pool(name="ropep", bufs=1) as rp, \
                     tc.tile_pool(name="rotp", bufs=3) as rotp, \
                     tc.tile_pool(name="rowp", bufs=4) as rowp:
                    cos_t = rp.tile([128, S], F32, name="cos_t")
                    nc.sync.dma_start(out=cos_t[:], in_=cos2[:])
                    sin_t = rp.tile([128, S], F32, name="sin_t")
                    nc.sync.dma_start(out=sin_t[:], in_=sins2[:])
                    wq_t = [wqp.tile([128, FQ], F32R, name=f"wq{i}")
                            for i in range(NT)]
                    wkv_t = [wqp.tile([128, 128], F32R, name=f"wkv{i}")
                             for i in range(NT)]
                    for sn in range(SC):
                        sl = slice(sn * 512, (sn + 1) * 512)
                        ssq = psq.tile([1, 512], F32, tag="ssq", name=f"ssq{sn}")
                        xrow = []
                        for i in range(NT):
                            xt = xsp.tile([128, 512], F32R, name=f"x_{sn}_{i}",
                                          tag="xs")
                            nc.sync.dma_start(
                                out=xt[:],
                                in_=f32r(xT[i * 128:(i + 1) * 128,
                                            sn * 512:(sn + 1) * 512]))
                            xrow.append(xt)
                            sq = sqp.tile([128, 512], F32R, tag="sq")
                            nc.vector.tensor_tensor(sq[:], xt[:], xt[:], OP.mult)
                            nc.tensor.matmul(ssq[:], f32r(ones_t[:]), sq[:],
                                             start=(i == 0), stop=(i == NT - 1))
                        if sn == 0:
                            for i in range(NT):
                                nc.sync.dma_start(
                                    out=wq_t[i][:],
                                    in_=f32r(wqT[i * 128:(i + 1) * 128, :]))
                                nc.sync.dma_start(
                                    out=wkv_t[i][:],
                                    in_=f32r(wkvT[i * 128:(i + 1) * 128, :]))
                        srow = rowp.tile([1, 512], F32, tag="srow")
                        nc.scalar.activation(srow[:], ssq[:], AF.Sqrt,
                                             bias=EPS, scale=1.0 / HID)
                        rrow = rowp.tile([1, 512], F32R, tag="rrow")
                        with nc.allow_low_precision(reason="f32r rstd bcast"):
                            nc.vector.reciprocal(rrow[:], srow[:])
                        rbp_ps = psq.tile([128, 512], F32, tag="ssq",
                                          name=f"rbcp{sn}")
                        nc.tensor.matmul(rbp_ps[:], ones_r[:], rrow[:],
                                         start=True, stop=True)
                        rbc = rowp.tile([128, 512], F32, tag="rbc",
                                        name=f"rbc{sn}")
                        nc.scalar.copy(rbc[:], rbp_ps[:])
                        for fc in range(2):
                            pst = ps.tile([128, 512], F32, tag="mm")
                            for i in range(NT):
                                nc.tensor.matmul(
                                    pst[:], wq_t[i][:, fc * 128:(fc + 1) * 128],
                                    xrow[i][:],
                                    start=(i == 0), stop=(i == NT - 1))
                            nc.vector.tensor_tensor(qT[fc][:, sl], f32r(pst[:]),
                                                    f32r(rbc[:]), OP.mult)
                        pst = ps.tile([128, 512], F32, tag="mm")
                        for i in range(NT):
                            nc.tensor.matmul(pst[:], wkv_t[i][:], xrow[i][:],
                                             start=(i == 0), stop=(i == NT - 1))
                        nc.vector.tensor_tensor(kvT[:, sl], f32r(pst[:]),
                                                f32r(rbc[:]), OP.mult)
                        # rope on this s-chunk
                        for tq in range(3):
                            tgt = qT[tq] if tq < 2 else kvT
                            hi = 128 if tq < 2 else 64
                            rot = rotp.tile([128, 512], F32R, tag="rot")
                            for b in range(0, hi, 64):
                                nc.vector.tensor_copy(
                                    rot[b:b + 32, :], tgt[b + 32:b + 64, sl])
                                nc.vector.tensor_copy(
                                    rot[b + 32:b + 64, :], tgt[b:b + 32, sl])
                            nc.vector.tensor_tensor(tgt[0:hi, sl], tgt[0:hi, sl],
                                                    f32r(cos_t[0:hi, sl]),
                                                    OP.mult)
                            nc.vector.tensor_tensor(rot[0:hi, :], rot[0:hi, :],
                                                    f32r(sin_t[0:hi, sl]),
                                                    OP.mult)
                            nc.vector.tensor_tensor(tgt[0:hi, sl], tgt[0:hi, sl],
                                                    rot[0:hi, :], OP.add)
                        nc.vector.tensor_copy(kkT[0:64, sl], kvT[0:64, sl])
                        nc.vector.tensor_copy(kkT[64:128, sl], kvT[0:64, sl])
                        vtmp = rotp.tile([64, 512], F32R, tag="vtmp")
                        nc.vector.tensor_copy(vtmp[:], kvT[64:128, sl])
                        for stl in range(4):
                            st = sn * 4 + stl
                            pst = ps.tile([128, 64], F32, tag="mm")
                            nc.tensor.transpose(
                                pst[:],
                                vtmp[:, stl * 128:(stl + 1) * 128].bitcast(F32),
                                ident[0:64, 0:64])
                            nc.scalar.copy(vN[:, st * 64:(st + 1) * 64], pst[:])

                # ---- attention ----
                with tc.tile_pool(name="mskp", bufs=1) as mp_, \
                     tc.tile_pool(name="pnp", bufs=4) as pnp, \
                     tc.tile_pool(name="expp", bufs=10) as expp, \
                     tc.tile_pool(name="rbp", bufs=3) as rbp, \
                     tc.tile_pool(name="rowp2", bufs=4) as rowp2:
                    mN = []
                    mT = []
                    for m in range(4):
                        t1 = mp_.tile([128, 512], F32, name=f"mN{m}")
                        nc.sync.dma_start(out=t1[:],
                                          in_=maskN[m * 128:(m + 1) * 128, :])
                        mN.append(t1)
                        t2 = mp_.tile([128, 512], F32, name=f"mT{m}")
                        nc.sync.dma_start(out=t2[:],
                                          in_=maskT[m * 128:(m + 1) * 128, :])
                        mT.append(t2)

                    for j in range(SC):
                        jsl = slice(j * 512, (j + 1) * 512)
                        # normal pass: attn_w rows + 1/sumexp
                        for i in range(4 * j, 4 * j + 4):
                            nchunks = i // 4 + 1
                            for h in range(NQ):
                                p = h // 2
                                hb = (h % 2) * 64
                                pn = pnp.tile([128, S], F32, tag="pn")
                                accs = rowp2.tile([128, 4], F32, tag="accs")
                                for n in range(nchunks):
                                    w = min(512, (i + 1) * 128 - n * 512)
                                    pst = ps.tile([128, 512], F32, tag="mm")
                                    nc.tensor.matmul(
                                        pst[:, 0:w],
                                        qT[p][hb:hb + 64, i * 128:(i + 1) * 128],
                                        kkT[hb:hb + 64, n * 512:n * 512 + w],
                                        start=True, stop=True,
                                        tile_position=(hb, 0))
                                    if n == i // 4:
                                        m0 = (i % 4) * 128
                                        nc.vector.tensor_tensor(
                                            pst[:, m0:w], pst[:, m0:w],
                                            mN[i % 4][:, m0:w], OP.add)
                                    nc.scalar.activation(
                                        pn[:, n * 512:n * 512 + w],
                                        pst[:, 0:w], AF.Exp,
                                        accum_out=accs[:, n:n + 1])
                                se = rowp2.tile([128, 1], F32, tag="se")
                                if nchunks == 1:
                                    nc.vector.reciprocal(se[:], accs[:, 0:1])
                                else:
                                    nc.vector.reduce_sum(se[:],
                                                         accs[:, 0:nchunks],
                                                         axis=AX.X)
                                    nc.vector.reciprocal(se[:], se[:])
                                wid = (i + 1) * 128
                                pnb = pnp.tile([128, S], BF16, tag="pnb")
                                nc.vector.tensor_scalar_mul(
                                    pnb[:, 0:wid], pn[:, 0:wid], se[:, 0:1])
                                nc.sync.dma_start(
                                    out=attn_w[h, i * 128:(i + 1) * 128, 0:wid],
                                    in_=pnb[:, 0:wid])
                                nc.sync.dma_start(
                                    out=rrow_dr[h, i * 128:(i + 1) * 128],
                                    in_=se[:, 0:1])
                        # transposed pass
                        rb = []
                        for pr in range(2):
                            t = rbp.tile([128, 512], F32, tag="rb")
                            nc.sync.dma_start(
                                out=t[0:64, :],
                                in_=rrow_dr[2 * pr, jsl].unsqueeze(0)
                                    .partition_broadcast(64))
                            nc.sync.dma_start(
                                out=t[64:128, :],
                                in_=rrow_dr[2 * pr + 1, jsl].unsqueeze(0)
                                    .partition_broadcast(64))
                            rb.append(t)
                        av = [psav.tile([128, 512], F32, tag="av",
                                        name=f"av{j}_{p2}") for p2 in range(2)]
                        nk = 4 * (j + 1)
                        for kc in range(nk):
                            et = []
                            for h in range(NQ):
                                hb = (h % 2) * 64
                                pst = ps.tile([128, 512], F32, tag="mm")
                                nc.tensor.matmul(
                                    pst[:],
                                    kkT[hb:hb + 64, kc * 128:(kc + 1) * 128],
                                    qT[h // 2][hb:hb + 64, jsl],
                                    start=True, stop=True,
                                    tile_position=(hb, 0))
                                if kc >= 4 * j:
                                    nc.vector.tensor_tensor(pst[:], pst[:],
                                                            mT[kc - 4 * j][:],
                                                            OP.add)
                                e = expp.tile([128, 512], BF16, tag="expT")
                                nc.scalar.activation(e[:], pst[:], AF.Exp)
                                et.append(e)
                            for pr in range(2):
                                for z in range(2):
                                    nc.tensor.matmul(
                                        av[pr][z * 64:(z + 1) * 64, :],
                                        vN[:, kc * 64:(kc + 1) * 64],
                                        et[2 * pr + z][:],
                                        start=(kc == 0), stop=(kc == nk - 1),
                                        tile_position=(0, z * 64))
                        for pr in range(2):
                            nc.vector.tensor_tensor(aoT[pr][j][:],
                                                    f32r(av[pr][:]),
                                                    f32r(rb[pr][:]), OP.mult)

                # ---- o-proj + AR1 ----
                with tc.tile_pool(name="wop", bufs=1) as wop, \
                     tc.tile_pool(name="obp", bufs=4) as obp:
                    wo_t = [wop.tile([128, HID], F32R, name=f"wo{i}")
                            for i in range(2)]
                    for i in range(2):
                        nc.sync.dma_start(out=wo_t[i][:],
                                          in_=f32r(woT[i * 128:(i + 1) * 128, :]))
                    for g in range(SC):
                        for ih in range(NT):
                            pst = ps.tile([128, 512], F32, tag="mm")
                            for fc in range(2):
                                nc.tensor.matmul(
                                    pst[:], wo_t[fc][:, ih * 128:(ih + 1) * 128],
                                    aoT[fc][g][:],
                                    start=(fc == 0), stop=(fc == 1))
                            ob = obp.tile([128, 512], BF16, tag="ob")
                            nc.scalar.copy(ob[:], pst[:])
                            nc.sync.dma_start(
                                out=ar1_in[g][ih * 128:(ih + 1) * 128, :],
                                in_=ob[:])
                        if os.environ.get("KERNEL_SIM_NOCOLL"):
                            nc.sync.dma_start(out=ar1_out[g][:],
                                              in_=ar1_in[g][:])
                        else:
                            nc.gpsimd.collective_compute(
                                "AllReduce", OP.add, replica_groups=RG,
                                ins=[ar1_in[g][:].opt()],
                                outs=[ar1_out[g][:].opt()])
                        nc.sync.dma_start(
                            out=o_attn.rearrange("h (g s) -> h g s", g=SC)[:, g, :],
                            in_=ar1_out[g][:])

            # ================= MLP super-phase =================
            with tc.tile_pool(name="actp", bufs=1) as actp:
                actT = [actp.tile([128, S], BF16, name=f"act{f}")
                        for f in range(8)]

                with tc.tile_pool(name="hp", bufs=1) as hp, \
                     tc.tile_pool(name="xt2p", bufs=2) as xt2p, \
                     tc.tile_pool(name="sq2p", bufs=3) as sq2p, \
                     tc.tile_pool(name="wmlp", bufs=3) as wmp, \
                     tc.tile_pool(name="ygp", bufs=3) as ygp, \
                     tc.tile_pool(name="rowp3", bufs=4) as rowp3:
                    hsb = [[hp.tile([128, 512], BF16, name=f"h{i}_{g}")
                            for g in range(SC)] for i in range(NT)]
                    rbc2 = []
                    for g in range(SC):
                        for i in range(NT):
                            xt2 = xt2p.tile([128, 512], F32, tag="xt2")
                            nc.sync.dma_start(
                                out=xt2[:],
                                in_=xT[i * 128:(i + 1) * 128,
                                       g * 512:(g + 1) * 512])
                            at2 = xt2p.tile([128, 512], BF16, tag="at2")
                            nc.sync.dma_start(
                                out=at2[:],
                                in_=ar1_out[g][i * 128:(i + 1) * 128, :])
                            nc.vector.tensor_tensor(hsb[i][g][:], xt2[:],
                                                    at2[:], OP.add)
                    for sn in range(SC):
                        sl = slice(sn * 512, (sn + 1) * 512)
                        ssq2 = psq.tile([1, 512], F32, tag="ssq",
                                        name=f"ssq2{sn}")
                        for i in range(NT):
                            sq = sq2p.tile([128, 512], F32R, tag="sq2")
                            nc.vector.tensor_tensor(sq[:], hsb[i][sn][:],
                                                    hsb[i][sn][:], OP.mult)
                            nc.tensor.matmul(ssq2[:], f32r(ones_t[:]), sq[:],
                                             start=(i == 0), stop=(i == NT - 1))
                        srow = rowp3.tile([1, 512], F32, tag="srow2")
                        nc.scalar.activation(srow[:], ssq2[:], AF.Sqrt,
                                             bias=EPS, scale=1.0 / HID)
                        rrow = rowp3.tile([1, 512], F32R, tag="rrow2")
                        with nc.allow_low_precision(reason="f32r rstd bcast"):
                            nc.vector.reciprocal(rrow[:], srow[:])
                        rb2ps = psq.tile([128, 512], F32, tag="ssq",
                                         name=f"rbc2p{sn}")
                        nc.tensor.matmul(rb2ps[:], ones_r[:], rrow[:],
                                         start=True, stop=True)
                        rb2 = rowp3.tile([128, 512], F32, tag="rbc2",
                                         name=f"rbc2_{sn}")
                        nc.scalar.copy(rb2[:], rb2ps[:])
                        rbc2.append(rb2)

                    gsb = {}
                    for fgrp in range(8):
                        for kind in range(2):
                            wsrc = wgT if kind == 0 else wuT
                            wt = wmp.tile([128, NT * 128], BF16, tag="wmlp")
                            nc.sync.dma_start(
                                out=wt[:].rearrange("p (t f) -> p t f", t=NT),
                                in_=wsrc[:, fgrp * 128:(fgrp + 1) * 128]
                                    .rearrange("(t p) f -> p t f", p=128))
                            for sn in range(SC):
                                sl = slice(sn * 512, (sn + 1) * 512)
                                pst = ps.tile([128, 512], F32, tag="mm")
                                for t in range(NT):
                                    nc.tensor.matmul(
                                        pst[:], wt[:, t * 128:(t + 1) * 128],
                                        hsb[t][sn][:],
                                        start=(t == 0), stop=(t == NT - 1))
                                if kind == 0:
                                    yg = ygp.tile([128, 512], F32, tag="yg")
                                    nc.vector.tensor_tensor(yg[:], pst[:],
                                                            rbc2[sn][:],
                                                            OP.mult)
                                    g2 = ygp.tile([128, 512], F32, tag="g2",
                                                  name=f"g2_{fgrp}_{sn}")
                                    nc.scalar.activation(g2[:], yg[:], AF.Silu)
                                    gsb[(fgrp, sn)] = g2
                                else:
                                    yu = ygp.tile([128, 512], F32, tag="yu")
                                    nc.vector.tensor_tensor(yu[:], pst[:],
                                                            rbc2[sn][:],
                                                            OP.mult)
                                    nc.vector.tensor_tensor(
                                        actT[fgrp][:, sl], yu[:],
                                        gsb[(fgrp, sn)][:], OP.mult)

                # ---- down-proj + AR2 ----
                with tc.tile_pool(name="wdp", bufs=1) as wdp, \
                     tc.tile_pool(name="dbp", bufs=3) as dbp:
                    wd_t = [wdp.tile([128, HID], BF16, name=f"wd{f}")
                            for f in range(8)]
                    for f in range(8):
                        nc.sync.dma_start(out=wd_t[f][:],
                                          in_=wdT[f * 128:(f + 1) * 128, :])
                    for sn in range(SC):
                        for ih in range(NT):
                            pst = ps.tile([128, 512], F32, tag="mm")
                            for f in range(8):
                                nc.tensor.matmul(
                                    pst[:], wd_t[f][:, ih * 128:(ih + 1) * 128],
                                    actT[f][:, sn * 512:(sn + 1) * 512],
                                    start=(f == 0), stop=(f == 7))
                            db = dbp.tile([128, 512], BF16, tag="db")
                            nc.scalar.copy(db[:], pst[:])
                            nc.sync.dma_start(
                                out=ar2_in[sn][ih * 128:(ih + 1) * 128, :],
                                in_=db[:])
                        if os.environ.get("KERNEL_SIM_NOCOLL"):
                            nc.sync.dma_start(out=ar2_out[sn][:],
                                              in_=ar2_in[sn][:])
                        else:
                            nc.gpsimd.collective_compute(
                                "AllReduce", OP.add, replica_groups=RG,
                                ins=[ar2_in[sn][:].opt()],
                                outs=[ar2_out[sn][:].opt()])
                        nc.sync.dma_start(
                            out=o_mlp.rearrange("h (g s) -> h g s", g=SC)[:, sn, :],
                            in_=ar2_out[sn][:])
    nc.finalize()
    return nc


def kernel(hidden_states, attention_mask, position_ids, wq, wk, wv, wo,
           w_gate, w_up, w_down, ln1_w, ln2_w, **kwargs):
    import ml_dtypes
    hs = np.asarray(hidden_states, dtype=np.float32)
    x = hs[0]                                   # [S, HID]
    xT = np.ascontiguousarray(x.T)
    am = np.asarray(attention_mask, dtype=np.float32)[0, 0]
    pos = np.asarray(position_ids)[0].astype(np.int64)

    inv_freq = 1.0 / (ROPE_THETA ** (np.arange(0, D, 2, dtype=np.float32) / D))
    freqs = np.outer(pos.astype(np.float32), inv_freq)
    emb = np.concatenate([freqs, freqs], axis=-1)
    cosT = np.cos(emb).T.astype(np.float32)
    sinT = np.sin(emb).T.astype(np.float32)
    sinsT = sinT.copy()
    sinsT[0:32] = -sinT[0:32]
    cos2 = np.ascontiguousarray(np.tile(cosT, (2, 1)))
    sins2 = np.ascontiguousarray(np.tile(sinsT, (2, 1)))

    maskN = np.ascontiguousarray(am[0:512, 0:512])
    maskTm = np.ascontiguousarray(am[0:512, 0:512].T)
    ones_hv = np.ones((128, 1), dtype=np.float32)

    ln1 = np.asarray(ln1_w, dtype=np.float32)
    ln2 = np.asarray(ln2_w, dtype=np.float32)
    wq = np.asarray(wq, dtype=np.float32)
    wk = np.asarray(wk, dtype=np.float32)
    wv = np.asarray(wv, dtype=np.float32)
    wo = np.asarray(wo, dtype=np.float32)
    w_gate = np.asarray(w_gate, dtype=np.float32)
    w_up = np.asarray(w_up, dtype=np.float32)
    w_down = np.asarray(w_down, dtype=np.float32)

    in_maps = []
    for c in range(NCORES):
        wq_s = (wq[c * FQ:(c + 1) * FQ, :] * ln1[None, :]) / np.sqrt(D)
        wk_s = wk[c * D:(c + 1) * D, :] * ln1[None, :]
        wv_s = wv[c * D:(c + 1) * D, :] * ln1[None, :]
        wkv_s = np.concatenate([wk_s, wv_s], axis=0)
        wo_s = wo[:, c * FQ:(c + 1) * FQ]
        wg_s = w_gate[c * INT_SH:(c + 1) * INT_SH, :] * ln2[None, :]
        wu_s = w_up[c * INT_SH:(c + 1) * INT_SH, :] * ln2[None, :]
        wd_s = w_down[:, c * INT_SH:(c + 1) * INT_SH]
        in_maps.append(dict(
            xT=xT,
            wqT=np.ascontiguousarray(wq_s.T.astype(np.float32)),
            wkvT=np.ascontiguousarray(wkv_s.T.astype(np.float32)),
            woT=np.ascontiguousarray(wo_s.T.astype(np.float32)),
            wgT=np.ascontiguousarray(wg_s.T).astype(ml_dtypes.bfloat16),
            wuT=np.ascontiguousarray(wu_s.T).astype(ml_dtypes.bfloat16),
            wdT=np.ascontiguousarray(wd_s.T).astype(ml_dtypes.bfloat16),
            cos2=cos2, sins2=sins2, maskN=maskN, maskT=maskTm,
            ones_in=ones_hv, ones_row=ones_hv[:, 0:1].T.copy(),
        ))

    if 'nc' not in _CACHED:
        _CACHED['nc'] = _build()
    res = run_bass_kernel_spmd(_CACHED['nc'], in_maps,
                               core_ids=list(range(NCORES)), **kwargs)
    _CACHED['last_results'] = res

    r0 = res.results[0]
    attn_sum = np.asarray(r0["o_attn"]).astype(np.float32).T
    mlp_sum = np.asarray(r0["o_mlp"]).astype(np.float32).T
    out = x + attn_sum + mlp_sum
    aw = np.concatenate([np.asarray(res.results[c]["attn_w"])[None]
                         for c in range(NCORES)], axis=1)
    return (out[None].astype(np.float32), aw.astype(np.float32))
